# revision 1
# baseline (speedup 1.0000x reference)
"""ConvCaps dynamic-routing kernel for 8 TRN2 NeuronCores.

Strategy (data-parallel over batch B=8, one batch element per core):
  - Grouped 3x3 conv (groups=D=32) done as one matmul per group per
    pixel-tile: stationary = im2col patches [72, npx], moving = weights
    [72, 512], PSUM out [npx, 512] -> u tile in SBUF laid out
    [px_partition, D, c, d].  No u traffic to DRAM at all.
  - 3 dynamic-routing iterations run on the Vector engine entirely
    in SBUF with px on partitions: softmax over d, s/a einsums as
    multiply + segmented tensor_reduce over D (resp. c).
  - Output s [px, (c,d)] is PE-transposed to [(c,d), px] and DMA'd out.
"""

import numpy as np
from contextlib import ExitStack

import concourse.bacc as bacc
import concourse.bass as bass
import concourse.tile as tile
import concourse.mybir as mybir
from concourse.bass_utils import run_bass_kernel_spmd
from concourse.masks import make_identity

F32 = mybir.dt.float32
AF = mybir.ActivationFunctionType

B = 8
C_IN, D_IN = 8, 32
C_OUT, D_OUT = 16, 32
KS = 3
H = W = 32
HO = WO = 30
NPX = HO * WO                 # 900 output pixels per batch element
KDIM = C_IN * KS * KS         # 72 = contraction dim of the conv matmul
CD = C_OUT * D_OUT            # 512 out-channels per group
ITERS = 3
P = 128
EPS = 1e-8
# pixel tiles = groups of output rows (30 px each); partition dim <= 128
ROW_TILES = [(0, 4), (4, 4), (8, 4), (12, 4), (16, 4), (20, 4), (24, 4), (28, 2)]
DCH = 8                       # D-chunk size for the einsum passes
NCH = D_IN // DCH


def _body(ctx, tc, xb, wt, b0, out, zero_prior):
    nc = tc.nc
    consts = ctx.enter_context(tc.tile_pool(name="consts", bufs=1))
    wpool = ctx.enter_context(tc.tile_pool(name="wpool", bufs=1))
    x9pool = ctx.enter_context(tc.tile_pool(name="x9pool", bufs=1))
    upool = ctx.enter_context(tc.tile_pool(name="upool", bufs=1))
    rpool = ctx.enter_context(tc.tile_pool(name="rpool", bufs=1))
    tmppool = ctx.enter_context(tc.tile_pool(name="tmppool", bufs=2))
    opool = ctx.enter_context(tc.tile_pool(name="opool", bufs=2))
    psum_c = ctx.enter_context(tc.tile_pool(name="psum_c", bufs=6, space="PSUM"))
    psum_t = ctx.enter_context(tc.tile_pool(name="psum_t", bufs=2, space="PSUM"))

    w_sb = wpool.tile([KDIM, D_IN * CD], F32)
    nc.sync.dma_start(w_sb[:], wt)
    ident = consts.tile([P, P], F32)
    make_identity(nc, ident)
    b0_sb = consts.tile([P, D_IN, D_OUT], F32)
    nc.sync.dma_start(b0_sb[:], b0)

    for (r0, nr) in ROW_TILES:
        npx = nr * WO
        pxs = slice(0, npx)

        # ---- im2col: 9 shifted window loads; partition k = (kh*3+kw)*8 + C
        x9 = x9pool.tile([KDIM, D_IN, 4, WO], F32, tag="x9")
        for kh in range(KS):
            for kw in range(KS):
                kk = kh * KS + kw
                for j in range(nr):
                    # per-row copy keeps both DMA access patterns <= 3 dims
                    nc.sync.dma_start(
                        x9[kk * C_IN:(kk + 1) * C_IN, :, j, :],
                        xb[:, :, r0 + kh + j, kw:kw + WO],
                    )

        # ---- grouped conv: one matmul per group, psum -> u_t on ScalarE
        # u_t layout (D, c, d); strided reduces measure faster than dense
        u_t = upool.tile([P, D_IN, C_OUT, D_OUT], F32, tag="u")
        for g in range(D_IN):
            pu = psum_c.tile([P, CD], F32, tag="pu")
            nc.tensor.matmul(
                pu[pxs, :],
                x9[:, g, 0:nr, :],
                w_sb[:, g * CD:(g + 1) * CD],
                start=True, stop=True,
            )
            nc.scalar.copy(u_t[pxs, g], pu[pxs, :])

        # ---- routing state tiles
        b_t = rpool.tile([P, D_IN, D_OUT], F32, tag="b")
        c_t = rpool.tile([P, D_IN, D_OUT], F32, tag="c")
        s_t = rpool.tile([P, C_OUT, D_OUT], F32, tag="s")
        sk_t = rpool.tile([P, C_OUT, D_OUT], F32, tag="sk")
        sq_t = rpool.tile([P, C_OUT, D_OUT], F32, tag="sq")
        v_t = rpool.tile([P, C_OUT, D_OUT], F32, tag="v")
        ak_t = rpool.tile([P, DCH, D_OUT], F32, tag="ak")
        n2_t = rpool.tile([P, D_OUT], F32, tag="n2")
        r_t = rpool.tile([P, D_OUT], F32, tag="r")
        f_t = rpool.tile([P, D_OUT], F32, tag="f")
        ssum = rpool.tile([P, D_IN], F32, tag="ssum")

        nc.scalar.copy(b_t[pxs], b0_sb[pxs])

        for it in range(ITERS):
            first = it == 0
            last = it == ITERS - 1
            uniform0 = first and zero_prior

            # softmax over d (no max-subtraction: logits are O(1) here)
            if not uniform0:
                nc.scalar.activation(c_t[pxs], b_t[pxs], AF.Exp)
                nc.vector.reduce_sum(ssum[pxs], c_t[pxs],
                                     axis=mybir.AxisListType.X)
                nc.vector.reciprocal(ssum[pxs], ssum[pxs])
                nc.vector.tensor_mul(
                    c_t[pxs], c_t[pxs],
                    ssum[pxs].unsqueeze(2).broadcast_to((npx, D_IN, D_OUT)))

            # s[c,d] = sum_D c[D,d] * u[D,c,d]   (chunked over D;
            # multiplies on GpSimd, segmented reduces on Vector)
            if uniform0:
                # c is uniform 1/32: one big reduce over all of D
                red_in = u_t[pxs].rearrange("p a b c -> p (b c) a")
                nc.vector.reduce_sum(s_t[pxs], red_in,
                                     axis=mybir.AxisListType.X)
                nc.vector.tensor_scalar_mul(s_t[pxs], s_t[pxs], 1.0 / D_IN)
            else:
                for k in range(NCH):
                    dk = slice(k * DCH, (k + 1) * DCH)
                    dst = s_t if k == 0 else sk_t
                    tmp = tmppool.tile([P, DCH, C_OUT, D_OUT], F32, tag="tmp")
                    nc.gpsimd.tensor_mul(
                        tmp[pxs], u_t[pxs, dk],
                        c_t[pxs, dk].unsqueeze(2)
                        .broadcast_to((npx, DCH, C_OUT, D_OUT)))
                    red_in = tmp[pxs].rearrange("p a b c -> p (b c) a")
                    nc.vector.reduce_sum(dst[pxs], red_in,
                                         axis=mybir.AxisListType.X)
                    if k > 0:
                        nc.vector.tensor_add(s_t[pxs], s_t[pxs], sk_t[pxs])

            if last:
                break

            # squash over c: v = s * n2 / ((1+n2) * sqrt(n2+eps))
            nc.scalar.square(sq_t[pxs], s_t[pxs])
            nc.vector.reduce_sum(n2_t[pxs], sq_t[pxs].transpose([0, 2, 1]),
                                 axis=mybir.AxisListType.X)
            nc.vector.tensor_scalar_add(r_t[pxs], n2_t[pxs], EPS)
            nc.scalar.activation(r_t[pxs], r_t[pxs], AF.Sqrt)
            nc.vector.tensor_scalar_add(f_t[pxs], n2_t[pxs], 1.0)
            nc.vector.tensor_mul(f_t[pxs], f_t[pxs], r_t[pxs])
            nc.vector.reciprocal(f_t[pxs], f_t[pxs])
            nc.vector.tensor_mul(f_t[pxs], f_t[pxs], n2_t[pxs])
            nc.vector.tensor_mul(
                v_t[pxs], s_t[pxs],
                f_t[pxs].unsqueeze(1).broadcast_to((npx, C_OUT, D_OUT)))

            # b[D,d] += sum_c u[D,c,d] * v[c,d]   (chunked over D)
            for k in range(NCH):
                dk = slice(k * DCH, (k + 1) * DCH)
                tmp = tmppool.tile([P, DCH, C_OUT, D_OUT], F32, tag="tmp")
                nc.gpsimd.tensor_mul(
                    tmp[pxs], u_t[pxs, dk],
                    v_t[pxs].unsqueeze(1)
                    .broadcast_to((npx, DCH, C_OUT, D_OUT)))
                nc.vector.reduce_sum(ak_t[pxs],
                                     tmp[pxs].transpose([0, 1, 3, 2]),
                                     axis=mybir.AxisListType.X)
                nc.vector.tensor_add(b_t[pxs, dk], b_t[pxs, dk], ak_t[pxs])

        # ---- write s out as [(c,d), px]: PE transpose in 128-row blocks
        s_flat = s_t[:].rearrange("p a b -> p (a b)")
        for blk in range(CD // P):
            pt = psum_t.tile([P, 120], F32, tag="pt")
            nc.tensor.transpose(
                pt[:, pxs], s_flat[pxs, blk * P:(blk + 1) * P],
                ident[pxs, pxs])
            ob = opool.tile([P, 120], F32, tag="ob")
            nc.scalar.copy(ob[:, pxs], pt[:, pxs])
            nc.sync.dma_start(
                out[blk * P:(blk + 1) * P, r0 * WO:r0 * WO + npx],
                ob[:, pxs])


_CACHE = {}


def _build(zero_prior: bool):
    key = ("v3", zero_prior)
    if key in _CACHE:
        return _CACHE[key]
    nc = bacc.Bacc("TRN2", target_bir_lowering=False, debug=False,
                   enable_asserts=True, num_devices=B)
    xb = nc.dram_tensor("xb", [C_IN, D_IN, H, W], F32,
                        kind="ExternalInput").ap()
    wt = nc.dram_tensor("wt", [KDIM, D_IN * CD], F32,
                        kind="ExternalInput").ap()
    b0 = nc.dram_tensor("b0", [P, D_IN, D_OUT], F32,
                        kind="ExternalInput").ap()
    out = nc.dram_tensor("out", [CD, NPX], F32, kind="ExternalOutput").ap()
    with tile.TileContext(nc) as tc:
        with ExitStack() as ctx:
            _body(ctx, tc, xb, wt, b0, out, zero_prior)
    nc.compile()
    _CACHE[key] = nc
    return nc


def _prep_inputs(x, conv_w, prior):
    # weights: rows (D,c,d) x (C,kh,kw) -> [k=(kh,kw,C), (D,c,d)]
    wt = conv_w.reshape(D_IN, C_OUT, D_OUT, C_IN, KS, KS)
    wt = np.ascontiguousarray(wt.transpose(4, 5, 3, 0, 1, 2)).reshape(KDIM, D_IN * CD)
    pb = np.broadcast_to(prior.reshape(D_IN, D_OUT), (P, D_IN, D_OUT))
    b0 = np.ascontiguousarray(pb).astype(np.float32)
    in_maps = [
        {"xb": np.ascontiguousarray(x[b]), "wt": wt, "b0": b0}
        for b in range(B)
    ]
    return in_maps


def kernel(x, conv_w, prior):
    x = np.asarray(x, dtype=np.float32)
    conv_w = np.asarray(conv_w, dtype=np.float32)
    prior = np.asarray(prior, dtype=np.float32)
    zero_prior = not np.any(prior)
    nc = _build(zero_prior)
    in_maps = _prep_inputs(x, conv_w, prior)
    res = run_bass_kernel_spmd(nc, in_maps, list(range(B)))
    outs = [res.results[b]["out"].reshape(C_OUT, D_OUT, HO, WO)
            for b in range(B)]
    return np.stack(outs, axis=0).astype(np.float32)



# revision 5
# speedup vs baseline: 2.5820x; 2.5820x over previous
"""ConvCaps dynamic-routing kernel for 8 TRN2 NeuronCores (v4).

Strategy (data-parallel over batch B=8, one batch element per core):
  - Grouped 3x3 conv (groups=D=32) in bf16: stationary = im2col patches
    [72, npx], moving = weights [72, 512] per group, PSUM fp32.
    u kept in SBUF as bf16 [px, D, c, d]; no u traffic to DRAM.
  - iter-0 s (uniform routing weights for zero prior) comes free from the
    TensorEngine: a second matmul per group accumulates sum_D u into one
    PSUM bank.
  - Routing einsums on the Vector engine in bf16 2x mode:
      mul: u * w_bcast (broadcast axis kept off the innermost dim)
      reduce: fold-tree of contiguous halves (bf16 tensor_tensor adds run
      2 elem/cycle; tensor_reduce is capped at 1).
  - softmax/squash in fp32; small ops on GpSimd/Scalar to keep Vector on
    the einsums; double-buffered u/x9 for cross-tile overlap.
"""

import numpy as np
from contextlib import ExitStack

import concourse.bacc as bacc
import concourse.bass as bass
import concourse.tile as tile
import concourse.mybir as mybir
from concourse.bass_utils import run_bass_kernel_spmd
from concourse.masks import make_identity

F32 = mybir.dt.float32
BF16 = mybir.dt.bfloat16
AF = mybir.ActivationFunctionType
AX = mybir.AxisListType

B = 8
C_IN, D_IN = 8, 32
C_OUT, D_OUT = 16, 32
KS = 3
H = W = 32
HO = WO = 30
NPX = HO * WO                 # 900 output pixels per batch element
KDIM = C_IN * KS * KS         # 72 = contraction dim of the conv matmul
CD = C_OUT * D_OUT            # 512 out-channels per group
ITERS = 3
P = 128
EPS = 1e-8
ROW_TILES = [(0, 4), (4, 4), (8, 4), (12, 4), (16, 4), (20, 4), (24, 4), (28, 2)]


def _squash(nc, v_dst, s_in, pxs, npx, rp, scale=1.0):
    """v_dst[bf16 [P,512]] = squash(s_in * scale) over c; s layout (c,d)."""
    sq = rp["sq"]
    n2 = rp["n2"]
    r = rp["r"]
    f = rp["f"]
    nc.scalar.activation(sq[pxs], s_in[pxs], AF.Square, scale=scale)
    # n2[d] = sum_c sq[c,d]: view (d inner-stride-1, c stride-32)
    sqv = sq[pxs].rearrange("p (c d) -> p c d", c=C_OUT).transpose([0, 2, 1])
    nc.vector.reduce_sum(n2[pxs], sqv, axis=AX.X)
    nc.vector.tensor_scalar_add(r[pxs], n2[pxs], EPS)
    nc.scalar.activation(r[pxs], r[pxs], AF.Sqrt)
    nc.vector.tensor_scalar_add(f[pxs], n2[pxs], 1.0)
    nc.vector.tensor_mul(f[pxs], f[pxs], r[pxs])
    nc.vector.reciprocal(f[pxs], f[pxs])
    nc.vector.tensor_mul(f[pxs], f[pxs], n2[pxs])
    # v = s * scale * f_bcast  (f over d, broadcast over c -> outer-0)
    sv = s_in[pxs].rearrange("p (c d) -> p c d", c=C_OUT)
    fb = f[pxs].unsqueeze(1).broadcast_to((npx, C_OUT, D_OUT))
    vv = v_dst[pxs].rearrange("p (c d) -> p c d", c=C_OUT)
    if scale != 1.0:
        nc.vector.tensor_mul(vv, sv, fb)
        nc.vector.tensor_scalar_mul(v_dst[pxs], v_dst[pxs], scale)
    else:
        nc.vector.tensor_mul(vv, sv, fb)


def _fold_D(nc, tmpf, tmp2f, dst, pxs):
    """dst[P,512] f32 = sum over D (axis of 32) of tmp [p,(D,c,d)] bf16."""
    nc.vector.tensor_add(tmp2f[pxs, 0:8192], tmpf[pxs, 0:8192], tmpf[pxs, 8192:16384])
    nc.vector.tensor_add(tmpf[pxs, 0:4096], tmp2f[pxs, 0:4096], tmp2f[pxs, 4096:8192])
    nc.vector.tensor_add(tmp2f[pxs, 0:2048], tmpf[pxs, 0:2048], tmpf[pxs, 2048:4096])
    nc.vector.tensor_add(tmpf[pxs, 0:1024], tmp2f[pxs, 0:1024], tmp2f[pxs, 1024:2048])
    nc.vector.tensor_add(dst[pxs], tmpf[pxs, 0:512], tmpf[pxs, 512:1024])


def _fold_c(nc, tmpf, tmp2f, dst, pxs, npx):
    """dst[P,1024] f32 = sum over c (middle 16) of tmp [p,(D,c,d)] bf16."""
    t0 = tmpf[pxs, 0:16384].rearrange("p (a x) -> p a x", a=D_IN, x=512)
    d1 = tmp2f[pxs, 0:8192].rearrange("p (a x) -> p a x", a=D_IN, x=256)
    nc.vector.tensor_add(d1, t0[:, :, 0:256], t0[:, :, 256:512])
    d2 = tmpf[pxs, 0:4096].rearrange("p (a x) -> p a x", a=D_IN, x=128)
    nc.vector.tensor_add(d2, d1[:, :, 0:128], d1[:, :, 128:256])
    d3 = tmp2f[pxs, 0:2048].rearrange("p (a x) -> p a x", a=D_IN, x=64)
    nc.vector.tensor_add(d3, d2[:, :, 0:64], d2[:, :, 64:128])
    d4 = dst[pxs].rearrange("p (a x) -> p a x", a=D_IN, x=D_OUT)
    nc.vector.tensor_add(d4, d3[:, :, 0:32], d3[:, :, 32:64])


def _body(ctx, tc, xb, wt, b0, c0, out, zero_prior):
    nc = tc.nc
    consts = ctx.enter_context(tc.tile_pool(name="consts", bufs=1))
    x9pool = ctx.enter_context(tc.tile_pool(name="x9pool", bufs=2))
    upool = ctx.enter_context(tc.tile_pool(name="upool", bufs=2))
    tmppool = ctx.enter_context(tc.tile_pool(name="tmppool", bufs=1))
    rpool = ctx.enter_context(tc.tile_pool(name="rpool", bufs=1))
    opool = ctx.enter_context(tc.tile_pool(name="opool", bufs=2))
    psum_c = ctx.enter_context(tc.tile_pool(name="psum_c", bufs=4, space="PSUM"))
    psum_s = ctx.enter_context(tc.tile_pool(name="psum_s", bufs=2, space="PSUM"))
    psum_t = ctx.enter_context(tc.tile_pool(name="psum_t", bufs=2, space="PSUM"))

    w_sb = consts.tile([KDIM, D_IN * CD], BF16)
    nc.sync.dma_start(w_sb[:], wt)
    ident = consts.tile([P, P], F32)
    make_identity(nc, ident)
    b0_sb = consts.tile([P, D_IN * D_OUT], F32)
    nc.sync.dma_start(b0_sb[:], b0)
    if not zero_prior:
        c0_sb = consts.tile([P, D_IN, D_OUT], BF16)
        nc.sync.dma_start(c0_sb[:], c0)

    xbv = xb.rearrange("c (d hw) -> c d hw", d=D_IN)

    for (r0, nr) in ROW_TILES:
        npx = nr * WO
        pxs = slice(0, npx)

        # ---- im2col: per-row 3-dim DMAs (30-wide packed rows)
        x9b = x9pool.tile([KDIM, D_IN, 4, 30], BF16, tag="x9")
        for kh in range(KS):
            for kw in range(KS):
                kk = kh * KS + kw
                for j in range(nr):
                    off = (r0 + kh + j) * W + kw
                    nc.sync.dma_start(
                        x9b[kk * C_IN:(kk + 1) * C_IN, :, j, :],
                        xbv[:, :, off:off + 30],
                    )

        # ---- grouped conv in bf16; ps0 accumulates sum_D u on the PE
        u_t = upool.tile([P, D_IN, C_OUT, D_OUT], BF16, tag="u")
        if zero_prior:
            ps0 = psum_s.tile([P, CD], F32, tag="ps0")
        for g in range(D_IN):
            stat = x9b[:, g, 0:nr, :]
            mov = w_sb[:, g * CD:(g + 1) * CD]
            pu = psum_c.tile([P, CD], F32, tag="pu")
            nc.tensor.matmul(pu[pxs], stat, mov, start=True, stop=True)
            if zero_prior:
                nc.tensor.matmul(ps0[pxs], stat, mov,
                                 start=(g == 0), stop=(g == D_IN - 1))
            nc.scalar.copy(u_t[pxs, g].rearrange("p c d -> p (c d)"), pu[pxs])

        # ---- routing state
        rp = {
            "b": rpool.tile([P, D_IN * D_OUT], F32, tag="b", name="rb"),
            "a": rpool.tile([P, D_IN * D_OUT], F32, tag="a", name="ra"),
            "e": rpool.tile([P, D_IN, D_OUT], F32, tag="e", name="re"),
            "c": rpool.tile([P, D_IN, D_OUT], BF16, tag="c", name="rc"),
            "s": rpool.tile([P, CD], F32, tag="s", name="rs"),
            "s0": rpool.tile([P, CD], F32, tag="s0", name="rs0"),
            "sq": rpool.tile([P, CD], F32, tag="sq", name="rsq"),
            "v": rpool.tile([P, CD], BF16, tag="v", name="rv"),
            "z": rpool.tile([P, D_IN], F32, tag="z", name="rz"),
            "n2": rpool.tile([P, D_OUT], F32, tag="n2", name="rn2"),
            "r": rpool.tile([P, D_OUT], F32, tag="r", name="rr"),
            "f": rpool.tile([P, D_OUT], F32, tag="f", name="rf"),
        }
        tmp = tmppool.tile([P, D_IN * CD], BF16, tag="tmp")
        tmp2 = tmppool.tile([P, D_IN * CD // 2], BF16, tag="tmp2")
        u4 = u_t[pxs]
        tmp4 = tmp[pxs].rearrange("p (a b c) -> p a b c", a=D_IN, b=C_OUT)
        b_t, a_t, s_t, v_t, c_t = rp["b"], rp["a"], rp["s"], rp["v"], rp["c"]

        for it in range(ITERS):
            first, last = it == 0, it == ITERS - 1

            # routing weights c for this iteration
            if first:
                if zero_prior:
                    # s0 directly from PE accumulation (c uniform = 1/32)
                    nc.scalar.mul(rp["s0"][pxs], ps0[pxs], 1.0 / D_IN)
                    s_cur = rp["s0"]
                else:
                    cb = c0_sb[pxs].unsqueeze(2).broadcast_to(
                        (npx, D_IN, C_OUT, D_OUT))
                    nc.vector.tensor_mul(tmp4, u4, cb)
                    _fold_D(nc, tmp, tmp2, rp["s0"], pxs)
                    s_cur = rp["s0"]
            else:
                # softmax over d: c = exp(b)/Z  (no max-sub; logits are O(1))
                ev = rp["e"]
                nc.scalar.activation(
                    ev[pxs].rearrange("p a b -> p (a b)"), b_t[pxs], AF.Exp)
                nc.vector.reduce_sum(rp["z"][pxs], ev[pxs], axis=AX.X)
                nc.vector.reciprocal(rp["z"][pxs], rp["z"][pxs])
                zb = rp["z"][pxs].unsqueeze(2).broadcast_to((npx, D_IN, D_OUT))
                nc.gpsimd.tensor_mul(c_t[pxs], ev[pxs], zb)
                # s = sum_D c * u
                cb = c_t[pxs].unsqueeze(2).broadcast_to((npx, D_IN, C_OUT, D_OUT))
                nc.vector.tensor_mul(tmp4, u4, cb)
                _fold_D(nc, tmp, tmp2, s_t, pxs)
                s_cur = s_t

            if last:
                break

            # v = squash(s)
            _squash(nc, v_t, s_cur, pxs, npx, rp)

            # a[D,d] = sum_c u * v_bcast;  b += a
            vb = v_t[pxs].rearrange("p (c d) -> p c d", c=C_OUT).unsqueeze(1)\
                .broadcast_to((npx, D_IN, C_OUT, D_OUT))
            nc.vector.tensor_mul(tmp4, u4, vb)
            _fold_c(nc, tmp, tmp2, a_t, pxs, npx)
            if first:
                nc.gpsimd.tensor_add(b_t[pxs], b0_sb[pxs], a_t[pxs])
            else:
                nc.gpsimd.tensor_add(b_t[pxs], b_t[pxs], a_t[pxs])

        # ---- write s out as [(c,d), px]: PE transpose in 128-row blocks
        for blk in range(CD // P):
            pt = psum_t.tile([P, 120], F32, tag="pt")
            nc.tensor.transpose(
                pt[:, pxs], s_t[pxs, blk * P:(blk + 1) * P], ident[pxs, pxs])
            ob = opool.tile([P, 120], F32, tag="ob")
            nc.scalar.copy(ob[:, pxs], pt[:, pxs])
            nc.sync.dma_start(
                out[blk * P:(blk + 1) * P, r0 * WO:r0 * WO + npx],
                ob[:, pxs])


_CACHE = {}


def _build(zero_prior: bool):
    key = ("v4", zero_prior)
    if key in _CACHE:
        return _CACHE[key]
    nc = bacc.Bacc("TRN2", target_bir_lowering=False, debug=False,
                   enable_asserts=True, num_devices=B)
    xb = nc.dram_tensor("xb", [C_IN, D_IN * H * W], BF16,
                        kind="ExternalInput").ap()
    wt = nc.dram_tensor("wt", [KDIM, D_IN * CD], BF16,
                        kind="ExternalInput").ap()
    b0 = nc.dram_tensor("b0", [P, D_IN * D_OUT], F32,
                        kind="ExternalInput").ap()
    if not zero_prior:
        c0 = nc.dram_tensor("c0", [P, D_IN, D_OUT], BF16,
                            kind="ExternalInput").ap()
    else:
        c0 = None
    out = nc.dram_tensor("out", [CD, NPX], F32, kind="ExternalOutput").ap()
    with tile.TileContext(nc) as tc:
        with ExitStack() as ctx:
            _body(ctx, tc, xb, wt, b0, c0, out, zero_prior)
    nc.compile()
    _CACHE[key] = nc
    return nc


def _prep_inputs(x, conv_w, prior):
    import ml_dtypes
    bf16 = ml_dtypes.bfloat16
    # weights: rows (D,c,d) x (C,kh,kw) -> [k=(kh,kw,C), (D,c,d)]
    wt = conv_w.reshape(D_IN, C_OUT, D_OUT, C_IN, KS, KS)
    wt = np.ascontiguousarray(wt.transpose(4, 5, 3, 0, 1, 2)).reshape(
        KDIM, D_IN * CD).astype(bf16)
    pb = np.broadcast_to(prior.reshape(D_IN * D_OUT), (P, D_IN * D_OUT))
    b0 = np.ascontiguousarray(pb).astype(np.float32)
    zero_prior = not np.any(prior)
    in_maps = []
    for b in range(B):
        m = {
            "xb": np.ascontiguousarray(x[b].reshape(C_IN, D_IN * H * W)).astype(bf16),
            "wt": wt,
            "b0": b0,
        }
        if not zero_prior:
            pl = prior.reshape(D_IN, D_OUT).astype(np.float64)
            e = np.exp(pl - pl.max(axis=1, keepdims=True))
            c0 = (e / e.sum(axis=1, keepdims=True)).astype(np.float32)
            m["c0"] = np.ascontiguousarray(
                np.broadcast_to(c0, (P, D_IN, D_OUT))).astype(bf16)
        in_maps.append(m)
    return in_maps


def kernel(x, conv_w, prior):
    x = np.asarray(x, dtype=np.float32)
    conv_w = np.asarray(conv_w, dtype=np.float32)
    prior = np.asarray(prior, dtype=np.float32)
    zero_prior = not np.any(prior)
    nc = _build(zero_prior)
    in_maps = _prep_inputs(x, conv_w, prior)
    res = run_bass_kernel_spmd(nc, in_maps, list(range(B)))
    outs = [res.results[b]["out"].reshape(C_OUT, D_OUT, HO, WO)
            for b in range(B)]
    return np.stack(outs, axis=0).astype(np.float32)


# revision 10
# speedup vs baseline: 2.6146x; 1.0126x over previous
"""ConvCaps dynamic-routing kernel for 8 TRN2 NeuronCores (v4).

Strategy (data-parallel over batch B=8, one batch element per core):
  - Grouped 3x3 conv (groups=D=32) in bf16: stationary = im2col patches
    [72, npx], moving = weights [72, 512] per group, PSUM fp32.
    u kept in SBUF as bf16 [px, D, c, d]; no u traffic to DRAM.
  - iter-0 s (uniform routing weights for zero prior) comes free from the
    TensorEngine: a second matmul per group accumulates sum_D u into one
    PSUM bank.
  - Routing einsums on the Vector engine in bf16 2x mode:
      mul: u * w_bcast (broadcast axis kept off the innermost dim)
      reduce: fold-tree of contiguous halves (bf16 tensor_tensor adds run
      2 elem/cycle; tensor_reduce is capped at 1).
  - softmax/squash in fp32; small ops on GpSimd/Scalar to keep Vector on
    the einsums; double-buffered u/x9 for cross-tile overlap.
"""

import numpy as np
from contextlib import ExitStack

import concourse.bacc as bacc
import concourse.bass as bass
import concourse.tile as tile
import concourse.mybir as mybir
from concourse.bass_utils import run_bass_kernel_spmd
from concourse.masks import make_identity

F32 = mybir.dt.float32
BF16 = mybir.dt.bfloat16
AF = mybir.ActivationFunctionType
AX = mybir.AxisListType

B = 8
C_IN, D_IN = 8, 32
C_OUT, D_OUT = 16, 32
KS = 3
H = W = 32
HO = WO = 30
NPX = HO * WO                 # 900 output pixels per batch element
KDIM = C_IN * KS * KS         # 72 = contraction dim of the conv matmul
CD = C_OUT * D_OUT            # 512 out-channels per group
ITERS = 3
P = 128
EPS = 1e-8
ROW_TILES = [(0, 4), (4, 4), (8, 4), (12, 4), (16, 4), (20, 4), (24, 4), (28, 2)]


def _squash(nc, v_dst, s_in, pxs, npx, rp, scale=1.0):
    """v_dst[bf16 [P,512]] = squash(s_in * scale) over c; s layout (c,d)."""
    sq = rp["sq"]
    n2 = rp["n2"]
    r = rp["r"]
    f = rp["f"]
    nc.scalar.activation(sq[pxs], s_in[pxs], AF.Square, scale=scale)
    # n2[d] = sum_c sq[c,d]: view (d inner-stride-1, c stride-32)
    sqv = sq[pxs].rearrange("p (c d) -> p c d", c=C_OUT).transpose([0, 2, 1])
    nc.vector.reduce_sum(n2[pxs], sqv, axis=AX.X)
    nc.vector.tensor_scalar_add(r[pxs], n2[pxs], EPS)
    nc.scalar.activation(r[pxs], r[pxs], AF.Sqrt)
    nc.vector.tensor_scalar_add(f[pxs], n2[pxs], 1.0)
    nc.vector.tensor_mul(f[pxs], f[pxs], r[pxs])
    nc.vector.reciprocal(f[pxs], f[pxs])
    nc.vector.tensor_mul(f[pxs], f[pxs], n2[pxs])
    # v = s * scale * f_bcast  (f over d, broadcast over c -> outer-0)
    sv = s_in[pxs].rearrange("p (c d) -> p c d", c=C_OUT)
    fb = f[pxs].unsqueeze(1).broadcast_to((npx, C_OUT, D_OUT))
    vv = v_dst[pxs].rearrange("p (c d) -> p c d", c=C_OUT)
    if scale != 1.0:
        nc.vector.tensor_mul(vv, sv, fb)
        nc.vector.tensor_scalar_mul(v_dst[pxs], v_dst[pxs], scale)
    else:
        nc.vector.tensor_mul(vv, sv, fb)


DH = 16  # d-half size: einsum passes run per d-half to shrink the tmp arena


def _pass_s(nc, tmppool, u_t, c_view, s_dst, pxs, npx):
    """s_dst[P,(c,d)] f32 = sum_D u * c_bcast; per d-half chunks.

    c_view: [P, D, d] (bf16) access pattern (full d)."""
    for h in range(2):
        ds = slice(h * DH, (h + 1) * DH)
        tmp = tmppool.tile([P, D_IN * C_OUT * DH], BF16, tag="tmp", name="tmps")
        tmp2 = tmppool.tile([P, D_IN * C_OUT * DH // 2], BF16, tag="tmp2",
                            name="tmp2s")
        t4 = tmp[pxs].rearrange("p (a b c) -> p a b c", a=D_IN, b=C_OUT)
        cb = c_view[pxs, :, ds].unsqueeze(2).broadcast_to((npx, D_IN, C_OUT, DH))
        nc.vector.tensor_mul(t4, u_t[pxs, :, :, ds], cb)
        # fold over D: 32 -> 1 on contiguous halves (8192..512 elems)
        nc.vector.tensor_add(tmp2[pxs, 0:4096], tmp[pxs, 0:4096], tmp[pxs, 4096:8192])
        nc.vector.tensor_add(tmp[pxs, 0:2048], tmp2[pxs, 0:2048], tmp2[pxs, 2048:4096])
        nc.vector.tensor_add(tmp2[pxs, 0:1024], tmp[pxs, 0:1024], tmp[pxs, 1024:2048])
        nc.vector.tensor_add(tmp[pxs, 0:512], tmp2[pxs, 0:512], tmp2[pxs, 512:1024])
        sd = s_dst[pxs].rearrange("p (c d) -> p c d", c=C_OUT)[:, :, ds]
        sh = tmp[pxs, 0:256].rearrange("p (c d) -> p c d", c=C_OUT)
        nc.vector.tensor_add(sd, sh, tmp[pxs, 256:512].rearrange(
            "p (c d) -> p c d", c=C_OUT))


def _pass_a(nc, tmppool, u_t, v_t, a_dst, pxs, npx):
    """a_dst[P,(D,d)] f32 = sum_c u * v_bcast; per d-half chunks."""
    for h in range(2):
        ds = slice(h * DH, (h + 1) * DH)
        tmp = tmppool.tile([P, D_IN * C_OUT * DH], BF16, tag="tmp", name="tmpa")
        tmp2 = tmppool.tile([P, D_IN * C_OUT * DH // 2], BF16, tag="tmp2",
                            name="tmp2a")
        t4 = tmp[pxs].rearrange("p (a b c) -> p a b c", a=D_IN, b=C_OUT)
        vb = v_t[pxs].rearrange("p (c d) -> p c d", c=C_OUT)[:, :, ds]\
            .unsqueeze(1).broadcast_to((npx, D_IN, C_OUT, DH))
        nc.vector.tensor_mul(t4, u_t[pxs, :, :, ds], vb)
        # fold over c: 16 -> 1; contiguous 128..16-elem runs per D block
        t0 = tmp[pxs].rearrange("p (a x) -> p a x", a=D_IN, x=C_OUT * DH)
        d1 = tmp2[pxs].rearrange("p (a x) -> p a x", a=D_IN, x=C_OUT * DH // 2)
        nc.vector.tensor_add(d1, t0[:, :, 0:128], t0[:, :, 128:256])
        d2 = t0[:, :, 0:64]
        nc.vector.tensor_add(d2, d1[:, :, 0:64], d1[:, :, 64:128])
        d3 = d1[:, :, 0:32]
        nc.vector.tensor_add(d3, d2[:, :, 0:32], d2[:, :, 32:64])
        ad = a_dst[pxs].rearrange("p (a d) -> p a d", a=D_IN)[:, :, ds]
        nc.vector.tensor_add(ad, d3[:, :, 0:DH], d3[:, :, DH:32])


def _body(ctx, tc, xb, wt, b0, c0, out, zero_prior):
    nc = tc.nc
    consts = ctx.enter_context(tc.tile_pool(name="consts", bufs=1))
    x9pool = ctx.enter_context(tc.tile_pool(name="x9pool", bufs=2))
    upool = ctx.enter_context(tc.tile_pool(name="upool", bufs=2))
    tmppool = ctx.enter_context(tc.tile_pool(name="tmppool", bufs=2))
    rpool = ctx.enter_context(tc.tile_pool(name="rpool", bufs=1))
    opool = ctx.enter_context(tc.tile_pool(name="opool", bufs=2))
    psum_c = ctx.enter_context(tc.tile_pool(name="psum_c", bufs=4, space="PSUM"))
    psum_s = ctx.enter_context(tc.tile_pool(name="psum_s", bufs=2, space="PSUM"))
    psum_t = ctx.enter_context(tc.tile_pool(name="psum_t", bufs=2, space="PSUM"))

    w_sb = consts.tile([KDIM, D_IN * CD], BF16)
    nc.sync.dma_start(w_sb[:], wt)
    ident = consts.tile([P, P], F32)
    make_identity(nc, ident)
    b0_sb = consts.tile([P, D_IN * D_OUT], F32)
    nc.sync.dma_start(b0_sb[:], b0)
    if not zero_prior:
        c0_sb = consts.tile([P, D_IN, D_OUT], BF16)
        nc.sync.dma_start(c0_sb[:], c0)

    xbv = xb.rearrange("c (d hw) -> c d hw", d=D_IN)

    for (r0, nr) in ROW_TILES:
        npx = nr * WO
        pxs = slice(0, npx)

        # ---- im2col: per-row 3-dim DMAs (30-wide packed rows)
        x9b = x9pool.tile([KDIM, D_IN, 4, 30], BF16, tag="x9")
        for kh in range(KS):
            for kw in range(KS):
                kk = kh * KS + kw
                for j in range(nr):
                    off = (r0 + kh + j) * W + kw
                    nc.sync.dma_start(
                        x9b[kk * C_IN:(kk + 1) * C_IN, :, j, :],
                        xbv[:, :, off:off + 30],
                    )

        # ---- grouped conv in bf16; ps0 accumulates sum_D u on the PE
        u_t = upool.tile([P, D_IN, C_OUT, D_OUT], BF16, tag="u")
        if zero_prior:
            ps0 = psum_s.tile([P, CD], F32, tag="ps0")
        for g in range(D_IN):
            stat = x9b[:, g, 0:nr, :]
            mov = w_sb[:, g * CD:(g + 1) * CD]
            pu = psum_c.tile([P, CD], F32, tag="pu")
            nc.tensor.matmul(pu[pxs], stat, mov, start=True, stop=True)
            if zero_prior:
                nc.tensor.matmul(ps0[pxs], stat, mov,
                                 start=(g == 0), stop=(g == D_IN - 1))
            nc.scalar.copy(u_t[pxs, g].rearrange("p c d -> p (c d)"), pu[pxs])

        # ---- routing state
        rp = {
            "b": rpool.tile([P, D_IN * D_OUT], F32, tag="b", name="rb"),
            "a": rpool.tile([P, D_IN * D_OUT], F32, tag="a", name="ra", bufs=2),
            "e": rpool.tile([P, D_IN, D_OUT], F32, tag="e", name="re", bufs=2),
            "c": rpool.tile([P, D_IN, D_OUT], BF16, tag="c", name="rc", bufs=2),
            "s": rpool.tile([P, CD], F32, tag="s", name="rs"),
            "s0": rpool.tile([P, CD], F32, tag="s0", name="rs0", bufs=2),
            "sq": rpool.tile([P, CD], F32, tag="sq", name="rsq", bufs=2),
            "v": rpool.tile([P, CD], BF16, tag="v", name="rv", bufs=2),
            "z": rpool.tile([P, D_IN], F32, tag="z", name="rz", bufs=2),
            "n2": rpool.tile([P, D_OUT], F32, tag="n2", name="rn2", bufs=2),
            "r": rpool.tile([P, D_OUT], F32, tag="r", name="rr", bufs=2),
            "f": rpool.tile([P, D_OUT], F32, tag="f", name="rf", bufs=2),
        }
        b_t, a_t, s_t, v_t, c_t = rp["b"], rp["a"], rp["s"], rp["v"], rp["c"]

        for it in range(ITERS):
            first, last = it == 0, it == ITERS - 1

            # routing weights c for this iteration
            if first:
                if zero_prior:
                    # s0 directly from PE accumulation (c uniform = 1/32)
                    nc.scalar.mul(rp["s0"][pxs], ps0[pxs], 1.0 / D_IN)
                    s_cur = rp["s0"]
                else:
                    _pass_s(nc, tmppool, u_t, c0_sb, rp["s0"], pxs, npx)
                    s_cur = rp["s0"]
            else:
                # softmax over d: c = exp(b)/Z  (no max-sub; logits are O(1))
                ev = rp["e"]
                nc.scalar.activation(
                    ev[pxs].rearrange("p a b -> p (a b)"), b_t[pxs], AF.Exp)
                nc.vector.reduce_sum(rp["z"][pxs], ev[pxs], axis=AX.X)
                nc.vector.reciprocal(rp["z"][pxs], rp["z"][pxs])
                zb = rp["z"][pxs].unsqueeze(2).broadcast_to((npx, D_IN, D_OUT))
                nc.gpsimd.tensor_mul(c_t[pxs], ev[pxs], zb)
                # s = sum_D c * u
                _pass_s(nc, tmppool, u_t, c_t, s_t, pxs, npx)
                s_cur = s_t

            if last:
                break

            # v = squash(s)
            _squash(nc, v_t, s_cur, pxs, npx, rp)

            # a[D,d] = sum_c u * v_bcast;  b += a
            _pass_a(nc, tmppool, u_t, v_t, a_t, pxs, npx)
            if first:
                nc.gpsimd.tensor_add(b_t[pxs], b0_sb[pxs], a_t[pxs])
            else:
                nc.gpsimd.tensor_add(b_t[pxs], b_t[pxs], a_t[pxs])

        # ---- write s out as [(c,d), px]: PE transpose in 128-row blocks
        for blk in range(CD // P):
            pt = psum_t.tile([P, 120], F32, tag="pt")
            nc.tensor.transpose(
                pt[:, pxs], s_t[pxs, blk * P:(blk + 1) * P], ident[pxs, pxs])
            ob = opool.tile([P, 120], F32, tag="ob")
            nc.scalar.copy(ob[:, pxs], pt[:, pxs])
            nc.sync.dma_start(
                out[blk * P:(blk + 1) * P, r0 * WO:r0 * WO + npx],
                ob[:, pxs])


_CACHE = {}


def _build(zero_prior: bool):
    key = ("v4", zero_prior)
    if key in _CACHE:
        return _CACHE[key]
    nc = bacc.Bacc("TRN2", target_bir_lowering=False, debug=False,
                   enable_asserts=True, num_devices=B)
    xb = nc.dram_tensor("xb", [C_IN, D_IN * H * W], BF16,
                        kind="ExternalInput").ap()
    wt = nc.dram_tensor("wt", [KDIM, D_IN * CD], BF16,
                        kind="ExternalInput").ap()
    b0 = nc.dram_tensor("b0", [P, D_IN * D_OUT], F32,
                        kind="ExternalInput").ap()
    if not zero_prior:
        c0 = nc.dram_tensor("c0", [P, D_IN, D_OUT], BF16,
                            kind="ExternalInput").ap()
    else:
        c0 = None
    out = nc.dram_tensor("out", [CD, NPX], F32, kind="ExternalOutput").ap()
    with tile.TileContext(nc) as tc:
        with ExitStack() as ctx:
            _body(ctx, tc, xb, wt, b0, c0, out, zero_prior)
    nc.compile()
    _CACHE[key] = nc
    return nc


def _prep_inputs(x, conv_w, prior):
    import ml_dtypes
    bf16 = ml_dtypes.bfloat16
    # weights: rows (D,c,d) x (C,kh,kw) -> [k=(kh,kw,C), (D,c,d)]
    wt = conv_w.reshape(D_IN, C_OUT, D_OUT, C_IN, KS, KS)
    wt = np.ascontiguousarray(wt.transpose(4, 5, 3, 0, 1, 2)).reshape(
        KDIM, D_IN * CD).astype(bf16)
    pb = np.broadcast_to(prior.reshape(D_IN * D_OUT), (P, D_IN * D_OUT))
    b0 = np.ascontiguousarray(pb).astype(np.float32)
    zero_prior = not np.any(prior)
    in_maps = []
    for b in range(B):
        m = {
            "xb": np.ascontiguousarray(x[b].reshape(C_IN, D_IN * H * W)).astype(bf16),
            "wt": wt,
            "b0": b0,
        }
        if not zero_prior:
            pl = prior.reshape(D_IN, D_OUT).astype(np.float64)
            e = np.exp(pl - pl.max(axis=1, keepdims=True))
            c0 = (e / e.sum(axis=1, keepdims=True)).astype(np.float32)
            m["c0"] = np.ascontiguousarray(
                np.broadcast_to(c0, (P, D_IN, D_OUT))).astype(bf16)
        in_maps.append(m)
    return in_maps


def kernel(x, conv_w, prior):
    x = np.asarray(x, dtype=np.float32)
    conv_w = np.asarray(conv_w, dtype=np.float32)
    prior = np.asarray(prior, dtype=np.float32)
    zero_prior = not np.any(prior)
    nc = _build(zero_prior)
    in_maps = _prep_inputs(x, conv_w, prior)
    res = run_bass_kernel_spmd(nc, in_maps, list(range(B)))
    outs = [res.results[b]["out"].reshape(C_OUT, D_OUT, HO, WO)
            for b in range(B)]
    return np.stack(outs, axis=0).astype(np.float32)


# revision 19
# speedup vs baseline: 2.6761x; 1.0235x over previous
"""ConvCaps dynamic-routing kernel for 8 TRN2 NeuronCores (v4).

Strategy (data-parallel over batch B=8, one batch element per core):
  - Grouped 3x3 conv (groups=D=32) in bf16: stationary = im2col patches
    [72, npx], moving = weights [72, 512] per group, PSUM fp32.
    u kept in SBUF as bf16 [px, D, c, d]; no u traffic to DRAM.
  - iter-0 s (uniform routing weights for zero prior) comes free from the
    TensorEngine: a second matmul per group accumulates sum_D u into one
    PSUM bank.
  - Routing einsums on the Vector engine in bf16 2x mode:
      mul: u * w_bcast (broadcast axis kept off the innermost dim)
      reduce: fold-tree of contiguous halves (bf16 tensor_tensor adds run
      2 elem/cycle; tensor_reduce is capped at 1).
  - softmax/squash in fp32; small ops on GpSimd/Scalar to keep Vector on
    the einsums; double-buffered u/x9 for cross-tile overlap.
"""

import numpy as np
from contextlib import ExitStack

import concourse.bacc as bacc
import concourse.bass as bass
import concourse.tile as tile
import concourse.mybir as mybir
from concourse.bass_utils import run_bass_kernel_spmd
from concourse.masks import make_identity

F32 = mybir.dt.float32
BF16 = mybir.dt.bfloat16
AF = mybir.ActivationFunctionType
AX = mybir.AxisListType

B = 8
C_IN, D_IN = 8, 32
C_OUT, D_OUT = 16, 32
KS = 3
H = W = 32
HO = WO = 30
NPX = HO * WO                 # 900 output pixels per batch element
KDIM = C_IN * KS * KS         # 72 = contraction dim of the conv matmul
CD = C_OUT * D_OUT            # 512 out-channels per group
ITERS = 3
P = 128
EPS = 1e-8
ROW_TILES = [(0, 4), (4, 4), (8, 4), (12, 4), (16, 4), (20, 4), (24, 4), (28, 2)]


def _squash(nc, v_dst, s_in, pxs, npx, rp, scale=1.0):
    """v_dst[bf16 [P,512]] = squash(s_in * scale) over c; s layout (c,d).

    s_in may live in PSUM (scale folded into n2 and f, so s is read raw)."""
    sq = rp["sq"]
    n2 = rp["n2"]
    r = rp["r"]
    f = rp["f"]
    from concourse.bass import MemorySpace
    if s_in.space == MemorySpace.PSUM:
        nc.scalar.activation(sq[pxs], s_in[pxs], AF.Square)
    else:
        nc.vector.tensor_mul(sq[pxs], s_in[pxs], s_in[pxs])
    # n2[d] = sum_c sq[c,d]: view (d inner-stride-1, c stride-32)
    sqv = sq[pxs].rearrange("p (c d) -> p c d", c=C_OUT).transpose([0, 2, 1])
    nc.vector.reduce_sum(n2[pxs], sqv, axis=AX.X)
    if scale != 1.0:
        nc.vector.tensor_scalar_mul(n2[pxs], n2[pxs], scale * scale)
    nc.vector.tensor_scalar_add(r[pxs], n2[pxs], EPS)
    nc.scalar.activation(r[pxs], r[pxs], AF.Sqrt)
    nc.vector.tensor_scalar_add(f[pxs], n2[pxs], 1.0)
    nc.vector.tensor_mul(f[pxs], f[pxs], r[pxs])
    nc.vector.reciprocal(f[pxs], f[pxs])
    nc.vector.tensor_mul(f[pxs], f[pxs], n2[pxs])
    if scale != 1.0:
        nc.vector.tensor_scalar_mul(f[pxs], f[pxs], scale)
    # v = s * (scale*f)_bcast  (f over d, broadcast over c -> outer-0)
    sv = s_in[pxs].rearrange("p (c d) -> p c d", c=C_OUT)
    fb = f[pxs].unsqueeze(1).broadcast_to((npx, C_OUT, D_OUT))
    vv = v_dst[pxs].rearrange("p (c d) -> p c d", c=C_OUT)
    nc.vector.tensor_mul(vv, sv, fb)


DH = 16  # d-half size: einsum passes run per d-half to shrink the tmp arena


def _pass_s(nc, tmppool, u_t, c_view, s_dst, pxs, npx):
    """s_dst[P,(c,d)] f32 = sum_D u * c_bcast; per d-half chunks.

    c_view: [P, D, d] (bf16) access pattern (full d)."""
    for h in range(2):
        ds = slice(h * DH, (h + 1) * DH)
        tmp = tmppool.tile([P, D_IN * C_OUT * DH], BF16, tag="tmp", name="tmps")
        tmp2 = tmppool.tile([P, D_IN * C_OUT * DH // 2], BF16, tag="tmp2",
                            name="tmp2s")
        t4 = tmp[pxs].rearrange("p (a b c) -> p a b c", a=D_IN, b=C_OUT)
        cb = c_view[pxs, :, ds].unsqueeze(2).broadcast_to((npx, D_IN, C_OUT, DH))
        nc.vector.tensor_mul(t4, u_t[pxs, :, :, ds], cb)
        # fold over D: 32 -> 1 on contiguous halves (8192..512 elems)
        nc.vector.tensor_add(tmp2[pxs, 0:4096], tmp[pxs, 0:4096], tmp[pxs, 4096:8192])
        nc.vector.tensor_add(tmp[pxs, 0:2048], tmp2[pxs, 0:2048], tmp2[pxs, 2048:4096])
        nc.vector.tensor_add(tmp2[pxs, 0:1024], tmp[pxs, 0:1024], tmp[pxs, 1024:2048])
        nc.vector.tensor_add(tmp[pxs, 0:512], tmp2[pxs, 0:512], tmp2[pxs, 512:1024])
        sd = s_dst[pxs].rearrange("p (c d) -> p c d", c=C_OUT)[:, :, ds]
        sh = tmp[pxs, 0:256].rearrange("p (c d) -> p c d", c=C_OUT)
        nc.vector.tensor_add(sd, sh, tmp[pxs, 256:512].rearrange(
            "p (c d) -> p c d", c=C_OUT))


def _pass_a(nc, tmppool, u_t, v_t, a_dst, pxs, npx):
    """a_dst[P,(D,d)] f32 = sum_c u * v_bcast; per d-half chunks."""
    for h in range(2):
        ds = slice(h * DH, (h + 1) * DH)
        tmp = tmppool.tile([P, D_IN * C_OUT * DH], BF16, tag="tmp", name="tmpa")
        tmp2 = tmppool.tile([P, D_IN * C_OUT * DH // 2], BF16, tag="tmp2",
                            name="tmp2a")
        t4 = tmp[pxs].rearrange("p (a b c) -> p a b c", a=D_IN, b=C_OUT)
        vb = v_t[pxs].rearrange("p (c d) -> p c d", c=C_OUT)[:, :, ds]\
            .unsqueeze(1).broadcast_to((npx, D_IN, C_OUT, DH))
        nc.vector.tensor_mul(t4, u_t[pxs, :, :, ds], vb)
        # fold over c: 16 -> 1; contiguous 128..16-elem runs per D block
        t0 = tmp[pxs].rearrange("p (a x) -> p a x", a=D_IN, x=C_OUT * DH)
        d1 = tmp2[pxs].rearrange("p (a x) -> p a x", a=D_IN, x=C_OUT * DH // 2)
        nc.vector.tensor_add(d1, t0[:, :, 0:128], t0[:, :, 128:256])
        d2 = t0[:, :, 0:64]
        nc.vector.tensor_add(d2, d1[:, :, 0:64], d1[:, :, 64:128])
        d3 = d1[:, :, 0:32]
        nc.vector.tensor_add(d3, d2[:, :, 0:32], d2[:, :, 32:64])
        ad = a_dst[pxs].rearrange("p (a d) -> p a d", a=D_IN)[:, :, ds]
        nc.vector.tensor_add(ad, d3[:, :, 0:DH], d3[:, :, DH:32])


def _body(ctx, tc, xb, wt, b0, c0, out, zero_prior):
    nc = tc.nc
    consts = ctx.enter_context(tc.tile_pool(name="consts", bufs=1))
    x9pool = ctx.enter_context(tc.tile_pool(name="x9pool", bufs=2))
    upool = ctx.enter_context(tc.tile_pool(name="upool", bufs=2))
    tmppool = ctx.enter_context(tc.tile_pool(name="tmppool", bufs=2))
    rpool = ctx.enter_context(tc.tile_pool(name="rpool", bufs=1))
    opool = ctx.enter_context(tc.tile_pool(name="opool", bufs=2))
    psum_c = ctx.enter_context(tc.tile_pool(name="psum_c", bufs=4, space="PSUM"))
    psum_s = ctx.enter_context(tc.tile_pool(name="psum_s", bufs=2, space="PSUM"))
    psum_t = ctx.enter_context(tc.tile_pool(name="psum_t", bufs=2, space="PSUM"))

    w_sb = consts.tile([KDIM, D_IN * CD], BF16)
    nc.sync.dma_start(w_sb[:], wt)
    ident = consts.tile([P, P], F32)
    make_identity(nc, ident)
    b0_sb = consts.tile([P, D_IN * D_OUT], F32)
    nc.sync.dma_start(b0_sb[:], b0)
    if not zero_prior:
        c0_sb = consts.tile([P, D_IN, D_OUT], BF16)
        nc.sync.dma_start(c0_sb[:], c0)

    xbv = xb.rearrange("c (d hw) -> c d hw", d=D_IN)

    # PE warm-up: ~4.3us of dummy matmuls so the HAM clock gate is at 8/8
    # by the time tile 0's conv issues (overlaps the initial DMAs).
    pdump = psum_c.tile([P, CD], F32, tag="pu", name="pdump")
    for _ in range(40):
        nc.tensor.matmul(pdump[:, 0:P], ident[:, :], ident[:, :],
                         start=True, stop=True)

    dma_engines = [nc.sync, nc.gpsimd, nc.sync, nc.gpsimd]
    n_dma = 0

    for (r0, nr) in ROW_TILES:
        npx = nr * WO
        pxs = slice(0, npx)

        # ---- im2col: per-row 3-dim DMAs (30-wide packed rows)
        x9b = x9pool.tile([KDIM, D_IN, 4, 30], BF16, tag="x9")
        for kh in range(KS):
            for kw in range(KS):
                kk = kh * KS + kw
                for j in range(nr):
                    off = (r0 + kh + j) * W + kw
                    dma_engines[n_dma % 4].dma_start(
                        x9b[kk * C_IN:(kk + 1) * C_IN, :, j, :],
                        xbv[:, :, off:off + 30],
                    )
                    n_dma += 1

        # ---- grouped conv in bf16; ps0 accumulates sum_D u on the PE
        u_t = upool.tile([P, D_IN, C_OUT, D_OUT], BF16, tag="u")
        if zero_prior:
            ps0 = psum_s.tile([P, CD], F32, tag="ps0")
        for g in range(D_IN):
            stat = x9b[:, g, 0:nr, :]
            mov = w_sb[:, g * CD:(g + 1) * CD]
            pu = psum_c.tile([P, CD], F32, tag="pu")
            nc.tensor.matmul(pu[pxs], stat, mov, start=True, stop=True)
            if zero_prior:
                nc.tensor.matmul(ps0[pxs], stat, mov,
                                 start=(g == 0), stop=(g == D_IN - 1))
            nc.scalar.copy(u_t[pxs, g].rearrange("p c d -> p (c d)"), pu[pxs])

        # ---- routing state
        rp = {
            "b": rpool.tile([P, D_IN * D_OUT], F32, tag="b", name="rb"),
            "a": rpool.tile([P, D_IN * D_OUT], F32, tag="a", name="ra", bufs=2),
            "e": rpool.tile([P, D_IN, D_OUT], F32, tag="e", name="re", bufs=2),
            "c": rpool.tile([P, D_IN, D_OUT], BF16, tag="c", name="rc", bufs=2),
            "s": rpool.tile([P, CD], F32, tag="s", name="rs"),
            "s0": rpool.tile([P, CD], F32, tag="s0", name="rs0", bufs=2),
            "sq": rpool.tile([P, CD], F32, tag="sq", name="rsq", bufs=2),
            "v": rpool.tile([P, CD], BF16, tag="v", name="rv", bufs=2),
            "z": rpool.tile([P, D_IN], F32, tag="z", name="rz", bufs=2),
            "n2": rpool.tile([P, D_OUT], F32, tag="n2", name="rn2", bufs=2),
            "r": rpool.tile([P, D_OUT], F32, tag="r", name="rr", bufs=2),
            "f": rpool.tile([P, D_OUT], F32, tag="f", name="rf", bufs=2),
        }
        b_t, a_t, s_t, v_t, c_t = rp["b"], rp["a"], rp["s"], rp["v"], rp["c"]

        for it in range(ITERS):
            first, last = it == 0, it == ITERS - 1

            # routing weights c for this iteration
            if first:
                if zero_prior:
                    # s0 straight from the PE accumulation (c uniform = 1/32);
                    # read PSUM directly, 1/32 folded into squash
                    s_cur = ps0
                else:
                    _pass_s(nc, tmppool, u_t, c0_sb, rp["s0"], pxs, npx)
                    s_cur = rp["s0"]
            else:
                # softmax over d: c = exp(b)/Z  (no max-sub; logits are O(1))
                ev = rp["e"]
                nc.scalar.activation(
                    ev[pxs].rearrange("p a b -> p (a b)"), b_t[pxs], AF.Exp)
                nc.vector.reduce_sum(rp["z"][pxs], ev[pxs], axis=AX.X)
                nc.vector.reciprocal(rp["z"][pxs], rp["z"][pxs])
                zb = rp["z"][pxs].unsqueeze(2).broadcast_to((npx, D_IN, D_OUT))
                nc.vector.tensor_mul(c_t[pxs], ev[pxs], zb)
                # s = sum_D c * u
                _pass_s(nc, tmppool, u_t, c_t, s_t, pxs, npx)
                s_cur = s_t

            if last:
                break

            # v = squash(s)
            _squash(nc, v_t, s_cur, pxs, npx, rp,
                    scale=(1.0 / D_IN) if (first and zero_prior) else 1.0)

            # a[D,d] = sum_c u * v_bcast;  b += a
            _pass_a(nc, tmppool, u_t, v_t, a_t, pxs, npx)
            if first:
                nc.vector.tensor_add(b_t[pxs], b0_sb[pxs], a_t[pxs])
            else:
                nc.vector.tensor_add(b_t[pxs], b_t[pxs], a_t[pxs])

        # ---- write s out as [(c,d), px]: PE transpose in 128-row blocks
        for blk in range(CD // P):
            pt = psum_t.tile([P, 120], F32, tag="pt")
            nc.tensor.transpose(
                pt[:, pxs], s_t[pxs, blk * P:(blk + 1) * P], ident[pxs, pxs])
            ob = opool.tile([P, 120], F32, tag="ob")
            nc.scalar.copy(ob[:, pxs], pt[:, pxs])
            nc.sync.dma_start(
                out[blk * P:(blk + 1) * P, r0 * WO:r0 * WO + npx],
                ob[:, pxs])


_CACHE = {}


def _build(zero_prior: bool):
    key = ("v4", zero_prior)
    if key in _CACHE:
        return _CACHE[key]
    nc = bacc.Bacc("TRN2", target_bir_lowering=False, debug=False,
                   enable_asserts=True, num_devices=B)
    xb = nc.dram_tensor("xb", [C_IN, D_IN * H * W], BF16,
                        kind="ExternalInput").ap()
    wt = nc.dram_tensor("wt", [KDIM, D_IN * CD], BF16,
                        kind="ExternalInput").ap()
    b0 = nc.dram_tensor("b0", [P, D_IN * D_OUT], F32,
                        kind="ExternalInput").ap()
    if not zero_prior:
        c0 = nc.dram_tensor("c0", [P, D_IN, D_OUT], BF16,
                            kind="ExternalInput").ap()
    else:
        c0 = None
    out = nc.dram_tensor("out", [CD, NPX], F32, kind="ExternalOutput").ap()
    with tile.TileContext(nc) as tc:
        with ExitStack() as ctx:
            _body(ctx, tc, xb, wt, b0, c0, out, zero_prior)
    nc.compile()
    _CACHE[key] = nc
    return nc


def _prep_inputs(x, conv_w, prior):
    import ml_dtypes
    bf16 = ml_dtypes.bfloat16
    # weights: rows (D,c,d) x (C,kh,kw) -> [k=(kh,kw,C), (D,c,d)]
    wt = conv_w.reshape(D_IN, C_OUT, D_OUT, C_IN, KS, KS)
    wt = np.ascontiguousarray(wt.transpose(4, 5, 3, 0, 1, 2)).reshape(
        KDIM, D_IN * CD).astype(bf16)
    pb = np.broadcast_to(prior.reshape(D_IN * D_OUT), (P, D_IN * D_OUT))
    b0 = np.ascontiguousarray(pb).astype(np.float32)
    zero_prior = not np.any(prior)
    in_maps = []
    for b in range(B):
        m = {
            "xb": np.ascontiguousarray(x[b].reshape(C_IN, D_IN * H * W)).astype(bf16),
            "wt": wt,
            "b0": b0,
        }
        if not zero_prior:
            pl = prior.reshape(D_IN, D_OUT).astype(np.float64)
            e = np.exp(pl - pl.max(axis=1, keepdims=True))
            c0 = (e / e.sum(axis=1, keepdims=True)).astype(np.float32)
            m["c0"] = np.ascontiguousarray(
                np.broadcast_to(c0, (P, D_IN, D_OUT))).astype(bf16)
        in_maps.append(m)
    return in_maps


def kernel(x, conv_w, prior):
    x = np.asarray(x, dtype=np.float32)
    conv_w = np.asarray(conv_w, dtype=np.float32)
    prior = np.asarray(prior, dtype=np.float32)
    zero_prior = not np.any(prior)
    nc = _build(zero_prior)
    in_maps = _prep_inputs(x, conv_w, prior)
    res = run_bass_kernel_spmd(nc, in_maps, list(range(B)))
    outs = [res.results[b]["out"].reshape(C_OUT, D_OUT, HO, WO)
            for b in range(B)]
    return np.stack(outs, axis=0).astype(np.float32)
